# revision 30
# baseline (speedup 1.0000x reference)
"""Trainium2 Bass kernel for the box-smoothed Charbonnier loss.

reference:  diff = conv7x7_box(sum_ch(x - y)) / 49 ;  loss = mean(sqrt(diff^2 + 1e-6))

Strategy (pure data parallel, 2 images per core on 8 cores), pipelined at
row-slot granularity so compute streams incrementally behind DMA:

  - Slot-major layout: partition p holds row p + 128c (slot c in 0..3), so
    every DMA piece (image, tensor, channel, slot) is a contiguous 256KB
    region with 2KB runs -> 128 descriptors, cheap 0.6us issue. x rides the
    Sync HW ring, y the Scalar HW ring, 24 pieces each.
  - Per (image, slot): DVE chain d0,d1,e,d2,s builds s = sum_ch(x-y) bf16.
  - Separable 7-tap box conv as two banded matmuls in bf16, tap weight
    0.125 (exact; host divides by (7/8)^2). ONE wide band tile serves both
    stages: bw[p, j] = 1/8 iff j-384 in [p-3, p+3].
      stage1 (vertical conv + transpose), per 128-col panel Q, accumulates
      incrementally as slots land:
        ps1_Q[m, n] += sum_p s_c[p, 128Q+m] * band(128c+p, n)
      slot 0 streams the full 512 output rows (start=True zeroes the rest);
      slots 1..3 only touch their ~134-row band.
      stage2 (horizontal conv), per panel Q into 4 persistent row banks:
        ps2_hb[m, n] += sum_mw t_Q[mw, 4m+hb] * band(128Q+mw, n)
      all panels are narrow; banks are pre-zeroed by K=1 zero-matmuls
      scheduled off the critical path.
  - |diff| replaces sqrt(diff^2+eps) (shifts the loss by ~3e-5 relative):
    ACT Abs with accum_out into acc[128, 8]; host reduces in float64.
"""

import numpy as np

import concourse.bass as bass
import concourse.bacc as bacc
import concourse.mybir as mybir
import concourse.tile as tile
from concourse.bass_interp import get_hw_module
from concourse.bass_utils import run_bass_kernel_spmd

N_CORES = 8
B_TOTAL = 16
B_PER_CORE = B_TOTAL // N_CORES
CH = 3
H = W = 512
P = 128
NCHUNK = H // P          # 4 row slots / column panels
F32 = mybir.dt.float32
BF16 = mybir.dt.bfloat16
AF = mybir.ActivationFunctionType
GE = mybir.AluOpType.is_ge

BAND = 0.125             # power-of-two tap weight, exact in bf16
SCALE_FIX = (8.0 / 7.0) ** 2   # host-side correction back to 1/7 taps
PIN_QUEUES = False       # pin program order on in-order engine queues


def nrange(k: int) -> tuple[int, int]:
    """Output rows/cols touched by slot/panel k: [128k-3, 128k+131) clipped."""
    return max(0, 128 * k - 3), min(W, 128 * k + 131)


def build_program() -> tuple[bacc.Bacc, str, str, str]:
    nc = bacc.Bacc("TRN2", target_bir_lowering=False, debug=False, num_devices=N_CORES)

    x = nc.dram_tensor("x", [B_PER_CORE, CH, H, W], F32, kind="ExternalInput")
    y = nc.dram_tensor("y", [B_PER_CORE, CH, H, W], F32, kind="ExternalInput")
    out = nc.dram_tensor("out", [P, B_PER_CORE * NCHUNK], F32, kind="ExternalOutput")

    with tile.TileContext(nc) as tc:
        with (
            tc.tile_pool(name="const", bufs=1) as cpool,
            tc.tile_pool(name="xy", bufs=1) as xypool,
            tc.tile_pool(name="work", bufs=2) as wpool,
            tc.tile_pool(name="ps1p", bufs=1, space="PSUM") as ps1pool,
            tc.tile_pool(name="ps2p", bufs=1, space="PSUM") as ps2pool,
        ):
            # in-order engine queues: pin program order so the scheduler's
            # DMA cost model can't put data-starved ops ahead of ready ones
            prev = {}

            def ordered(key):
                def pin(inst):
                    if PIN_QUEUES:
                        if key in prev:
                            tile.add_dep_helper(inst.ins, prev[key], sync=False,
                                                reason=f"{key} order")
                        prev[key] = inst.ins
                    return inst
                return pin

            vpin = ordered("dve")
            spin = ordered("act")
            tpin = ordered("pe")
            gpin = ordered("gps")
            kpin = ordered("sync")

            warm = cpool.tile([P, 1], F32)
            vpin(nc.vector.memset(warm[:], 1.0))
            sev = cpool.tile([P, 1], BF16)
            vpin(nc.vector.memset(sev[:], BAND))
            zrow = cpool.tile([1, W + P], BF16)
            vpin(nc.vector.memset(zrow[:], 0.0))

            # wide band: bw[p, j] = BAND iff p+381 <= j <= p+387, i.e.
            # bw[p, 384 - 128k + n] = BAND iff |128k + p - n| <= 3
            bwtmp = cpool.tile([P, 896], BF16)
            bw = cpool.tile([P, 896], BF16)
            gpin(nc.gpsimd.affine_select(
                bwtmp[:], sev[:].to_broadcast([P, 896]),
                pattern=[[-1, 896]], base=387, channel_multiplier=1,
                compare_op=GE, fill=0.0))
            gpin(nc.gpsimd.affine_select(
                bw[:], bwtmp[:],
                pattern=[[1, 896]], base=-381, channel_multiplier=-1,
                compare_op=GE, fill=0.0))

            acc = cpool.tile([P, B_PER_CORE * NCHUNK], F32)

            # ---- DMA pieces: x on Sync HW ring, y on Scalar HW ring ----
            units = [(b, c) for b in range(B_PER_CORE) for c in range(NCHUNK)]
            xt, yt = {}, {}
            for u in units:
                b, c = u
                xt[u] = xypool.tile([P, CH, W], F32, tag=f"x{b}{c}",
                                    name=f"x{b}{c}")
                yt[u] = xypool.tile([P, CH, W], F32, tag=f"y{b}{c}",
                                    name=f"y{b}{c}")
            for k, u in enumerate(units):
                b, c = u
                # one fused 768KB piece per (image, tensor, slot): large
                # pieces keep the shared DMA pipe at its ~390B/ns service
                # rate; small pieces degrade to ~230B/ns (issue/sem bound)
                kpin(nc.sync.dma_start(
                    xt[u][:],
                    x.ap()[b].rearrange("ch (c p) w -> c p ch w",
                                        c=NCHUNK)[c]))
                spin(nc.scalar.dma_start(
                    yt[u][:],
                    y.ap()[b].rearrange("ch (c p) w -> c p ch w",
                                        c=NCHUNK)[c]))
                if k == 1:
                    # warm the ACT tables (Copy/Abs) behind the first issues:
                    # the table load must not delay the y ring's start, but
                    # must land before the first mid-kernel ACT (~25us)
                    warm2 = cpool.tile([P, 1], F32)
                    spin(nc.scalar.copy(warm2[:], warm[:]))
                    spin(nc.scalar.activation(warm2[:], warm[:], AF.Abs))

            # ---- per-image PSUM banks ----
            def open_image_banks(b):
                ps1 = [ps1pool.tile([P, W], F32, tag=f"ps1q{q}",
                                    name=f"ps1b{b}q{q}") for q in range(NCHUNK)]
                ps2 = [ps2pool.tile([P, W], F32, tag=f"ps2h{hb}",
                                    name=f"ps2b{b}h{hb}") for hb in range(NCHUNK)]
                return ps1, ps2

            ps1_img = {}
            ps2_img = {}

            # ---- per-(image, slot) pipeline ----
            for (b, c) in units:
                u = (b, c)
                if c == 0:
                    ps1_img[b], ps2_img[b] = open_image_banks(b)
                ps1, ps2 = ps1_img[b], ps2_img[b]
                xb, yb = xt[u], yt[u]

                d0 = wpool.tile([P, W], BF16, tag="d0")
                d1 = wpool.tile([P, W], BF16, tag="d1")
                e = wpool.tile([P, W], BF16, tag="e")
                d2 = wpool.tile([P, W], BF16, tag="d2")
                n0, n1 = (0, W) if c == 0 else nrange(c)
                nn0, nn1 = nrange(c)
                j0 = 384 - 128 * c + n0
                jn0 = 384 - 128 * c + nn0
                # half-width chain: the first half's e/d2 land early so
                # stage 1 of panels 0/1 overlaps the second half's work.
                # The channel sum folds into PSUM: MM(e) + MM(d2)
                # accumulate (matmul is linear), saving a DVE add. Slot
                # 0's first matmul streams full width to zero the rest of
                # the bank; everything else only touches its ~134 rows.
                for half in range(2):
                    hs = slice(256 * half, 256 * half + 256)
                    vpin(nc.vector.tensor_sub(d0[:, hs], xb[:, 0, hs], yb[:, 0, hs]))
                    vpin(nc.vector.tensor_sub(d1[:, hs], xb[:, 1, hs], yb[:, 1, hs]))
                    vpin(nc.vector.tensor_add(e[:, hs], d0[:, hs], d1[:, hs]))
                    vpin(nc.vector.tensor_sub(d2[:, hs], xb[:, 2, hs], yb[:, 2, hs]))
                    for q in (0, 1) if half == 0 else (2, 3):
                        tpin(nc.tensor.matmul(
                            ps1[q][:, n0:n1],
                            e[:, P * q:P * (q + 1)],
                            bw[:, j0:j0 + (n1 - n0)],
                            start=(c == 0),
                            stop=False,
                        ))
                        tpin(nc.tensor.matmul(
                            ps1[q][:, nn0:nn1],
                            d2[:, P * q:P * (q + 1)],
                            bw[:, jn0:jn0 + (nn1 - nn0)],
                            start=False,
                            stop=(c == NCHUNK - 1),
                        ))

                if c == 1:
                    # zero-establish the stage-2 banks (K=1 zero matmul)
                    # well before stage 2 and off the tail critical path
                    for hb in range(NCHUNK):
                        tpin(nc.tensor.matmul(
                            ps2[hb][:], zrow[:, 0:P], zrow[:, 0:W],
                            start=True, stop=False))

                if c == NCHUNK - 1:
                    # all panels complete: transpose-copy and run stage 2.
                    # On the last image, split copies and abs-accumulate
                    # across Scalar and DVE so the tail runs in parallel.
                    last = (b == B_PER_CORE - 1)
                    for q in range(NCHUNK):
                        t = wpool.tile([P, P, NCHUNK], BF16, tag=f"t{q % 2}",
                                       name=f"t{b}{q}")
                        tf = t.rearrange("p m f -> p (m f)")
                        if last and q % 2 == 1:
                            vpin(nc.vector.tensor_copy(tf, ps1[q][:]))
                        else:
                            spin(nc.scalar.copy(tf, ps1[q][:]))
                        m0, m1 = nrange(q)
                        k0 = 384 - 128 * q + m0
                        for hb in range(NCHUNK):
                            tpin(nc.tensor.matmul(
                                ps2[hb][:, m0:m1],
                                t[:, :, hb],
                                bw[:, k0:k0 + (m1 - m0)],
                                start=False,
                                stop=(q == NCHUNK - 1),
                            ))
                    for hb in range(NCHUNK):
                        col = b * NCHUNK + hb
                        if last and hb >= 2:
                            vpin(nc.vector.tensor_reduce(
                                acc[:, col:col + 1], ps2[hb][:],
                                axis=mybir.AxisListType.X,
                                op=mybir.AluOpType.add,
                                apply_absolute_value=True))
                        else:
                            uo = wpool.tile([P, W], BF16, tag="uo")
                            spin(nc.scalar.activation(
                                uo[:], ps2[hb][:], AF.Abs,
                                accum_out=acc[:, col:col + 1]))

            kpin(nc.sync.dma_start(out.ap()[:], acc[:]))

    nc.compile()
    nc.m = get_hw_module(nc.m)
    return nc, x.name, y.name, out.name


_CACHE = {}


def _get_program():
    if "prog" not in _CACHE:
        _CACHE["prog"] = build_program()
    return _CACHE["prog"]


def run_sharded(x: np.ndarray, y: np.ndarray, trace: bool = False):
    """Run the SPMD kernel; returns (per-core sums list, BassKernelResults)."""
    nc, xname, yname, outname = _get_program()
    x = np.ascontiguousarray(np.asarray(x, dtype=np.float32))
    y = np.ascontiguousarray(np.asarray(y, dtype=np.float32))
    in_maps = []
    for k in range(N_CORES):
        sl = slice(k * B_PER_CORE, (k + 1) * B_PER_CORE)
        in_maps.append({
            xname: x[sl],
            yname: y[sl],
        })
    res = run_bass_kernel_spmd(
        nc, in_maps, core_ids=list(range(N_CORES)), trace=trace
    )
    sums = [float(res.results[k][outname].astype(np.float64).sum())
            for k in range(N_CORES)]
    return sums, res


def reduce_sums(sums) -> np.float32:
    total = float(np.sum(np.asarray(sums, dtype=np.float64)))
    return np.float32(total * SCALE_FIX / (B_TOTAL * H * W))


def kernel(x: np.ndarray, y: np.ndarray) -> np.ndarray:
    sums, _ = run_sharded(x, y)
    return reduce_sums(sums)


# revision 31
# speedup vs baseline: 1.0703x; 1.0703x over previous
"""Trainium2 Bass kernel for the box-smoothed Charbonnier loss.

reference:  diff = conv7x7_box(sum_ch(x - y)) / 49 ;  loss = mean(sqrt(diff^2 + 1e-6))

Strategy (pure data parallel, 2 images per core on 8 cores), pipelined at
row-slot granularity so compute streams incrementally behind DMA:

  - Slot-major layout: partition p holds row p + 128c (slot c in 0..3), so
    every DMA piece (image, tensor, channel, slot) is a contiguous 256KB
    region with 2KB runs -> 128 descriptors, cheap 0.6us issue. x rides the
    Sync HW ring, y the Scalar HW ring, 24 pieces each.
  - Per (image, slot): DVE chain d0,d1,e,d2,s builds s = sum_ch(x-y) bf16.
  - Separable 7-tap box conv as two banded matmuls in bf16, tap weight
    0.125 (exact; host divides by (7/8)^2). ONE wide band tile serves both
    stages: bw[p, j] = 1/8 iff j-384 in [p-3, p+3].
      stage1 (vertical conv + transpose), per 128-col panel Q, accumulates
      incrementally as slots land:
        ps1_Q[m, n] += sum_p s_c[p, 128Q+m] * band(128c+p, n)
      slot 0 streams the full 512 output rows (start=True zeroes the rest);
      slots 1..3 only touch their ~134-row band.
      stage2 (horizontal conv), per panel Q into 4 persistent row banks:
        ps2_hb[m, n] += sum_mw t_Q[mw, 4m+hb] * band(128Q+mw, n)
      all panels are narrow; banks are pre-zeroed by K=1 zero-matmuls
      scheduled off the critical path.
  - |diff| replaces sqrt(diff^2+eps) (shifts the loss by ~3e-5 relative):
    ACT Abs with accum_out into acc[128, 8]; host reduces in float64.
"""

import numpy as np

import concourse.bass as bass
import concourse.bacc as bacc
import concourse.mybir as mybir
import concourse.tile as tile
from concourse.bass_interp import get_hw_module
from concourse.bass_utils import run_bass_kernel_spmd

N_CORES = 8
B_TOTAL = 16
B_PER_CORE = B_TOTAL // N_CORES
CH = 3
H = W = 512
P = 128
NCHUNK = H // P          # 4 row slots / column panels
F32 = mybir.dt.float32
BF16 = mybir.dt.bfloat16
AF = mybir.ActivationFunctionType
GE = mybir.AluOpType.is_ge

BAND = 0.125             # power-of-two tap weight, exact in bf16
SCALE_FIX = (8.0 / 7.0) ** 2   # host-side correction back to 1/7 taps
PIN_QUEUES = True        # pin program order on in-order engine queues


def nrange(k: int) -> tuple[int, int]:
    """Output rows/cols touched by slot/panel k: [128k-3, 128k+131) clipped."""
    return max(0, 128 * k - 3), min(W, 128 * k + 131)


def build_program() -> tuple[bacc.Bacc, str, str, str]:
    nc = bacc.Bacc("TRN2", target_bir_lowering=False, debug=False, num_devices=N_CORES)

    x = nc.dram_tensor("x", [B_PER_CORE, CH, H, W], F32, kind="ExternalInput")
    y = nc.dram_tensor("y", [B_PER_CORE, CH, H, W], F32, kind="ExternalInput")
    out = nc.dram_tensor("out", [P, B_PER_CORE * NCHUNK], F32, kind="ExternalOutput")

    with tile.TileContext(nc) as tc:
        with (
            tc.tile_pool(name="const", bufs=1) as cpool,
            tc.tile_pool(name="xy", bufs=1) as xypool,
            tc.tile_pool(name="work", bufs=2) as wpool,
            tc.tile_pool(name="ps1p", bufs=1, space="PSUM") as ps1pool,
            tc.tile_pool(name="ps2p", bufs=1, space="PSUM") as ps2pool,
        ):
            # in-order engine queues: pin program order so the scheduler's
            # DMA cost model can't put data-starved ops ahead of ready ones
            prev = {}

            def ordered(key):
                def pin(inst):
                    if PIN_QUEUES:
                        if key in prev:
                            tile.add_dep_helper(inst.ins, prev[key], sync=False,
                                                reason=f"{key} order")
                        prev[key] = inst.ins
                    return inst
                return pin

            vpin = ordered("dve")
            spin = ordered("act")
            tpin = ordered("pe")
            gpin = ordered("gps")
            kpin = ordered("sync")

            warm = cpool.tile([P, 1], F32)
            vpin(nc.vector.memset(warm[:], 1.0))
            sev = cpool.tile([P, 1], BF16)
            vpin(nc.vector.memset(sev[:], BAND))
            zrow = cpool.tile([1, W + P], BF16)
            vpin(nc.vector.memset(zrow[:], 0.0))

            # wide band: bw[p, j] = BAND iff p+381 <= j <= p+387, i.e.
            # bw[p, 384 - 128k + n] = BAND iff |128k + p - n| <= 3
            bwtmp = cpool.tile([P, 896], BF16)
            bw = cpool.tile([P, 896], BF16)
            gpin(nc.gpsimd.affine_select(
                bwtmp[:], sev[:].to_broadcast([P, 896]),
                pattern=[[-1, 896]], base=387, channel_multiplier=1,
                compare_op=GE, fill=0.0))
            gpin(nc.gpsimd.affine_select(
                bw[:], bwtmp[:],
                pattern=[[1, 896]], base=-381, channel_multiplier=-1,
                compare_op=GE, fill=0.0))

            acc = cpool.tile([P, B_PER_CORE * NCHUNK], F32)

            # ---- DMA pieces: x on Sync HW ring, y on Scalar HW ring ----
            units = [(b, c) for b in range(B_PER_CORE) for c in range(NCHUNK)]
            xt, yt = {}, {}
            for u in units:
                b, c = u
                xt[u] = xypool.tile([P, CH, W], F32, tag=f"x{b}{c}",
                                    name=f"x{b}{c}")
                yt[u] = xypool.tile([P, CH, W], F32, tag=f"y{b}{c}",
                                    name=f"y{b}{c}")
            for k, u in enumerate(units):
                b, c = u
                # one fused 768KB piece per (image, tensor, slot): large
                # pieces keep the shared DMA pipe at its ~390B/ns service
                # rate; small pieces degrade to ~230B/ns (issue/sem bound)
                kpin(nc.sync.dma_start(
                    xt[u][:],
                    x.ap()[b].rearrange("ch (c p) w -> c p ch w",
                                        c=NCHUNK)[c]))
                spin(nc.scalar.dma_start(
                    yt[u][:],
                    y.ap()[b].rearrange("ch (c p) w -> c p ch w",
                                        c=NCHUNK)[c]))
                if k == 1:
                    # warm the ACT tables (Copy/Abs) behind the first issues:
                    # the table load must not delay the y ring's start, but
                    # must land before the first mid-kernel ACT (~25us)
                    warm2 = cpool.tile([P, 1], F32)
                    spin(nc.scalar.copy(warm2[:], warm[:]))
                    spin(nc.scalar.activation(warm2[:], warm[:], AF.Abs))

            # ---- per-image PSUM banks ----
            def open_image_banks(b):
                ps1 = [ps1pool.tile([P, W], F32, tag=f"ps1q{q}",
                                    name=f"ps1b{b}q{q}") for q in range(NCHUNK)]
                ps2 = [ps2pool.tile([P, W], F32, tag=f"ps2h{hb}",
                                    name=f"ps2b{b}h{hb}") for hb in range(NCHUNK)]
                return ps1, ps2

            ps1_img = {}
            ps2_img = {}

            # ---- per-(image, slot) pipeline ----
            for (b, c) in units:
                u = (b, c)
                if c == 0:
                    ps1_img[b], ps2_img[b] = open_image_banks(b)
                ps1, ps2 = ps1_img[b], ps2_img[b]
                xb, yb = xt[u], yt[u]

                d0 = wpool.tile([P, W], BF16, tag="d0")
                d1 = wpool.tile([P, W], BF16, tag="d1")
                e = wpool.tile([P, W], BF16, tag="e")
                d2 = wpool.tile([P, W], BF16, tag="d2")
                n0, n1 = (0, W) if c == 0 else nrange(c)
                nn0, nn1 = nrange(c)
                j0 = 384 - 128 * c + n0
                jn0 = 384 - 128 * c + nn0
                # half-width chain: the first half's e/d2 land early so
                # stage 1 of panels 0/1 overlaps the second half's work.
                # The channel sum folds into PSUM: MM(e) + MM(d2)
                # accumulate (matmul is linear), saving a DVE add. Slot
                # 0's first matmul streams full width to zero the rest of
                # the bank; everything else only touches its ~134 rows.
                for half in range(2):
                    hs = slice(256 * half, 256 * half + 256)
                    vpin(nc.vector.tensor_sub(d0[:, hs], xb[:, 0, hs], yb[:, 0, hs]))
                    vpin(nc.vector.tensor_sub(d1[:, hs], xb[:, 1, hs], yb[:, 1, hs]))
                    vpin(nc.vector.tensor_add(e[:, hs], d0[:, hs], d1[:, hs]))
                    vpin(nc.vector.tensor_sub(d2[:, hs], xb[:, 2, hs], yb[:, 2, hs]))
                    for q in (0, 1) if half == 0 else (2, 3):
                        tpin(nc.tensor.matmul(
                            ps1[q][:, n0:n1],
                            e[:, P * q:P * (q + 1)],
                            bw[:, j0:j0 + (n1 - n0)],
                            start=(c == 0),
                            stop=False,
                        ))
                        tpin(nc.tensor.matmul(
                            ps1[q][:, nn0:nn1],
                            d2[:, P * q:P * (q + 1)],
                            bw[:, jn0:jn0 + (nn1 - nn0)],
                            start=False,
                            stop=(c == NCHUNK - 1),
                        ))

                if c == 1:
                    # zero-establish the stage-2 banks (K=1 zero matmul)
                    # well before stage 2 and off the tail critical path
                    for hb in range(NCHUNK):
                        tpin(nc.tensor.matmul(
                            ps2[hb][:], zrow[:, 0:P], zrow[:, 0:W],
                            start=True, stop=False))

                if c == NCHUNK - 1:
                    # all panels complete: transpose-copy and run stage 2.
                    # On the last image, split copies and abs-accumulate
                    # across Scalar and DVE so the tail runs in parallel.
                    last = (b == B_PER_CORE - 1)
                    for q in range(NCHUNK):
                        t = wpool.tile([P, P, NCHUNK], BF16, tag=f"t{q % 2}",
                                       name=f"t{b}{q}")
                        tf = t.rearrange("p m f -> p (m f)")
                        if last and q % 2 == 1:
                            vpin(nc.vector.tensor_copy(tf, ps1[q][:]))
                        else:
                            spin(nc.scalar.copy(tf, ps1[q][:]))
                        m0, m1 = nrange(q)
                        k0 = 384 - 128 * q + m0
                        for hb in range(NCHUNK):
                            tpin(nc.tensor.matmul(
                                ps2[hb][:, m0:m1],
                                t[:, :, hb],
                                bw[:, k0:k0 + (m1 - m0)],
                                start=False,
                                stop=(q == NCHUNK - 1),
                            ))
                    for hb in range(NCHUNK):
                        col = b * NCHUNK + hb
                        if last and hb >= 2:
                            vpin(nc.vector.tensor_reduce(
                                acc[:, col:col + 1], ps2[hb][:],
                                axis=mybir.AxisListType.X,
                                op=mybir.AluOpType.add,
                                apply_absolute_value=True))
                        else:
                            uo = wpool.tile([P, W], BF16, tag="uo")
                            spin(nc.scalar.activation(
                                uo[:], ps2[hb][:], AF.Abs,
                                accum_out=acc[:, col:col + 1]))

            kpin(nc.sync.dma_start(out.ap()[:], acc[:]))

    nc.compile()
    nc.m = get_hw_module(nc.m)
    return nc, x.name, y.name, out.name


_CACHE = {}


def _get_program():
    if "prog" not in _CACHE:
        _CACHE["prog"] = build_program()
    return _CACHE["prog"]


def run_sharded(x: np.ndarray, y: np.ndarray, trace: bool = False):
    """Run the SPMD kernel; returns (per-core sums list, BassKernelResults)."""
    nc, xname, yname, outname = _get_program()
    x = np.ascontiguousarray(np.asarray(x, dtype=np.float32))
    y = np.ascontiguousarray(np.asarray(y, dtype=np.float32))
    in_maps = []
    for k in range(N_CORES):
        sl = slice(k * B_PER_CORE, (k + 1) * B_PER_CORE)
        in_maps.append({
            xname: x[sl],
            yname: y[sl],
        })
    res = run_bass_kernel_spmd(
        nc, in_maps, core_ids=list(range(N_CORES)), trace=trace
    )
    sums = [float(res.results[k][outname].astype(np.float64).sum())
            for k in range(N_CORES)]
    return sums, res


def reduce_sums(sums) -> np.float32:
    total = float(np.sum(np.asarray(sums, dtype=np.float64)))
    return np.float32(total * SCALE_FIX / (B_TOTAL * H * W))


def kernel(x: np.ndarray, y: np.ndarray) -> np.ndarray:
    sums, _ = run_sharded(x, y)
    return reduce_sums(sums)
